# revision 7
# baseline (speedup 1.0000x reference)
"""Causal multi-head attention (B=4, T=2048, D=1024, H=16, HD=64) on 8
Trainium2 NeuronCores.

Sharding: data-parallel over batch (4) x tensor-parallel over heads (2
groups of 8). Each core runs the same Bass program on its own input
slices; the host sums the two tensor-parallel partial projections per
batch and adds b_proj (plus the exact bv@wp correction: the V bias
commutes through softmax since sum p = 1).

v3: all matmul operands bf16 (measured on HW: bf16 matmul = 1 cycle
per output column regardless of contraction depth; f32r is ~5% slower
with 2x costlier LDWEIGHTS; fp8 DoubleRow gives no per-column speedup
and fails the 2e-2 gate on high-logit rows). Per-core dataflow:

  xbT [D,T] bf16 (host pre-transposed), w{q,k,v}b [D,DL] bf16
  Q^T,K^T = w-stationary bf16 matmuls -> bf16 [128, T] feature-major
            (+ per-feature bias on DVE)
  V       = xT-stationary bf16 matmuls -> bf16 token-major [128, 8*65]
            tiles (ones col 64 of each 65-wide head block)
  S^T     = K^T-block-stationary bf16 matmuls (contraction hd=64, two
            heads row-packed in the PE array), causal via partial-N
            matmuls + affine_select on diagonal blocks
  P       = exp(S/8) on ScalarE -> bf16 et [128, 1024] (2 heads)
  O^T,den = V|1-stationary bf16 matmuls accumulated over key blocks
            (ones column -> softmax denominator in PSUM row 64)
  out     = O^T * (1/den) via reciprocal_approx on the PSUM den row +
            DRAM-bounce partition broadcast
  yT      = w_proj-stationary bf16 matmuls -> [D, T] f32 partial

Attention runs chunk-major (c outer, head-pair inner) and the output
projection for chunk c is emitted right after its last head-pair, so
proj overlaps attention of chunk c+1 instead of serializing at the end.
"""

import numpy as np
import ml_dtypes

import concourse.bass as bass
import concourse.bacc as bacc
import concourse.mybir as mybir
import concourse.tile as tile
from concourse.bass_utils import run_bass_kernel_spmd

F32 = mybir.dt.float32
BF16 = mybir.dt.bfloat16
BFNP = ml_dtypes.bfloat16
AF = mybir.ActivationFunctionType
ALU = mybir.AluOpType

B, T, D = 4, 2048, 1024
H, HD = 16, 64
NH = 8          # heads per core
DL = NH * HD    # 512 local qkv feature dim
PAIRS = NH // 2
CH = T // 512   # 4 chunks of 512 tokens
KT = T // 128   # 16 tk blocks / token tiles
VW = 65         # V columns per head incl. ones column


def build(nc: bass.Bass):
    xbT = nc.declare_dram_parameter("xbT", [D, T], BF16, isOutput=False)
    wqb = nc.declare_dram_parameter("wqb", [D, DL], BF16, isOutput=False)
    wkb = nc.declare_dram_parameter("wkb", [D, DL], BF16, isOutput=False)
    wvb = nc.declare_dram_parameter("wvb", [D, DL], BF16, isOutput=False)
    bq = nc.declare_dram_parameter("bq", [DL], F32, isOutput=False)
    bk = nc.declare_dram_parameter("bk", [DL], F32, isOutput=False)
    wp = nc.declare_dram_parameter("wp", [DL, D], BF16, isOutput=False)
    ones8 = nc.declare_dram_parameter("ones8", [128, 8], BF16, isOutput=False)
    yT = nc.declare_dram_parameter("yT", [D, T], F32, isOutput=True)

    with tile.TileContext(nc) as tc:
        with (
            tc.tile_pool(name="persist", bufs=1) as persist,
            tc.tile_pool(name="dram", bufs=4, space="DRAM") as dram,
        ):
            # -------- persistent tiles --------
            qkT = [persist.tile([128, T], BF16, tag=f"qk{i}", name=f"qk{i}")
                   for i in range(8)]
            v_sb = [persist.tile([128, NH * VW], BF16, tag=f"v{i}",
                                 name=f"v{i}") for i in range(KT)]
            osb = [persist.tile([128, T], BF16, tag=f"o{i}", name=f"o{i}")
                   for i in range(4)]
            bq_sb = persist.tile([128, 4], F32, tag="bq", name="bq_sb")
            bk_sb = persist.tile([128, 4], F32, tag="bk", name="bk_sb")
            nc.sync.dma_start(
                out=bq_sb, in_=bq[:].rearrange("(a p) -> p a", p=128)
            )
            nc.sync.dma_start(
                out=bk_sb, in_=bk[:].rearrange("(a p) -> p a", p=128)
            )

            # ================ phase 1: QKV projections ================
            with (
                nc.named_scope("qkv"),
                tc.tile_pool(name="ph1sb", bufs=1) as ph1sb,
                tc.tile_pool(name="ph1ps", bufs=6, space="PSUM") as ph1ps,
            ):
                wts = {}
                for wname, w_in in (("q", wqb), ("k", wkb), ("v", wvb)):
                    wts[wname] = []
                    for k in range(8):
                        t_ = ph1sb.tile([128, DL], BF16, tag=f"w{wname}{k}",
                                        name=f"w{wname}{k}")
                        nc.sync.dma_start(
                            out=t_, in_=w_in[128 * k : 128 * k + 128, :]
                        )
                        wts[wname].append(t_)
                for c in range(CH):
                    cs = slice(512 * c, 512 * c + 512)
                    xt = []
                    for k in range(8):
                        t_ = ph1sb.tile([128, 512], BF16, tag="xt", bufs=16,
                                        name="xt")
                        nc.sync.dma_start(
                            out=t_, in_=xbT[128 * k : 128 * k + 128, cs]
                        )
                        xt.append(t_)
                    # V token-major (first: attention waits on all of V)
                    for t4 in range(4):
                        tt = 4 * c + t4
                        acc = ph1ps.tile([128, 512], F32, tag="ps", name="acc")
                        for k in range(8):
                            nc.tensor.matmul(
                                acc,
                                xt[k][:, 128 * t4 : 128 * t4 + 128],
                                wts["v"][k],
                                start=(k == 0),
                                stop=(k == 7),
                            )
                        vdst = v_sb[tt].rearrange("p (h c) -> p h c", c=VW)
                        # ones in col 64 of each 65-wide head block
                        nc.sync.dma_start(
                            out=vdst[:, :, HD], in_=ones8[:, 0:NH]
                        )
                        nc.vector.tensor_copy(
                            vdst[:, :, 0:HD],
                            acc[:].rearrange("p (h c) -> p h c", c=HD),
                        )
                    # Q^T then K^T feature-major bf16, + per-feature bias
                    for wname, b_sb, obase in (("q", bq_sb, 0),
                                               ("k", bk_sb, 4)):
                        for n in range(4):
                            acc = ph1ps.tile([128, 512], F32, tag="ps",
                                             name="acc")
                            for k in range(8):
                                nc.tensor.matmul(
                                    acc,
                                    wts[wname][k][:, 128 * n : 128 * n + 128],
                                    xt[k],
                                    start=(k == 0), stop=(k == 7),
                                )
                            nc.vector.tensor_scalar_add(
                                out=qkT[obase + n][:, cs],
                                in0=acc,
                                scalar1=b_sb[:, n : n + 1],
                            )

            # prefetch proj weights (overlaps attention)
            at2sb_cm = tc.tile_pool(name="at2sb", bufs=1)
            at2sb = at2sb_cm.__enter__()
            wp_sb = [at2sb.tile([128, D], BF16, tag=f"wp{k}",
                                name=f"wp{k}") for k in range(4)]
            for k in range(4):
                nc.sync.dma_start(
                    out=wp_sb[k], in_=wp[128 * k : 128 * k + 128, :]
                )

            # ===== phase 2+3: attention (chunk-major) + interleaved proj ====
            with (
                nc.named_scope("attn"),
                tc.tile_pool(name="atps", bufs=1, space="PSUM") as atps,
            ):
                for c in range(CH):
                    qs = slice(512 * c, 512 * c + 512)
                    for g2 in range(PAIRS):
                        qt, kt = qkT[g2], qkT[4 + g2]
                        ha, hb = 2 * g2, 2 * g2 + 1
                        av_a = atps.tile([VW, 512], F32, tag="av", bufs=3,
                                         name="av_a")
                        av_b = atps.tile([VW, 512], F32, tag="av", bufs=3,
                                         name="av_b")
                        nb = 4 * (c + 1)
                        for b in range(nb):
                            diag = (b // 4 == c)
                            off = 128 * (b - 4 * c) if diag else 0
                            bs = slice(128 * b, 128 * b + 128)
                            strip = atps.tile([128, 1024], F32, tag="strip",
                                              bufs=2, name="strip")
                            et = at2sb.tile([128, 1024], BF16, tag="exp",
                                            bufs=4, name="et")
                            nc.tensor.matmul(
                                strip[:, off:512],
                                kt[0:64, bs],
                                qt[0:64, 512 * c + off : 512 * c + 512],
                                start=True, stop=True,
                            )
                            nc.tensor.matmul(
                                strip[:, 512 + off : 1024],
                                kt[64:128, bs],
                                qt[64:128, 512 * c + off : 512 * c + 512],
                                start=True, stop=True,
                            )
                            if off == 0:
                                nc.scalar.activation(
                                    et[:, 0:1024], strip[:, 0:1024],
                                    AF.Exp, scale=0.125,
                                )
                            else:
                                # one instr over both heads' valid regions:
                                # cols [off,512) and [512+off,1024)
                                w_ = 512 - off
                                src_ap = bass.AP(
                                    tensor=strip.tensor,
                                    offset=strip.offset + off,
                                    ap=[list(strip.ap[0]), [512, 2], [1, w_]],
                                )
                                dst_ap = bass.AP(
                                    tensor=et.tensor,
                                    offset=et.offset + off,
                                    ap=[list(et.ap[0]), [512, 2], [1, w_]],
                                )
                                nc.scalar.activation(dst_ap, src_ap, AF.Exp,
                                                     scale=0.125)
                            if diag:
                                sel = bass.AP(
                                    tensor=et.tensor,
                                    offset=et.offset + off,
                                    ap=[list(et.ap[0]), [512, 2], [1, 128]],
                                )
                                nc.gpsimd.affine_select(
                                    out=sel,
                                    in_=sel,
                                    compare_op=ALU.is_ge,
                                    fill=0.0,
                                    base=0,
                                    pattern=[[0, 2], [1, 128]],
                                    channel_multiplier=-1,
                                )
                            nc.tensor.matmul(
                                av_a[:, off:512],
                                v_sb[b][:, VW * ha : VW * ha + VW],
                                et[:, off:512],
                                start=(b == 0), stop=(b == nb - 1),
                            )
                            nc.tensor.matmul(
                                av_b[:, off:512],
                                v_sb[b][:, VW * hb : VW * hb + VW],
                                et[:, 512 + off : 1024],
                                start=(b == 0), stop=(b == nb - 1),
                            )
                        for h, av in ((0, av_a), (1, av_b)):
                            rec = at2sb.tile([1, 512], F32, tag="rec", bufs=4,
                                             name="rec")
                            scr = at2sb.tile([1, 512], F32, tag="scr", bufs=4,
                                             name="scr")
                            nc.vector.reciprocal_approx_accurate(
                                rec, av[64:65, :], scratch=scr
                            )
                            rd = dram.tile([1, 512], F32, tag="rd", bufs=4,
                                           name="rd")
                            nc.sync.dma_start(out=rd, in_=rec)
                            bc = at2sb.tile([64, 512], F32, tag="bc", bufs=4,
                                           name="bc")
                            nc.sync.dma_start(
                                out=bc,
                                in_=bass.AP(tensor=rd.tensor, offset=rd.offset,
                                            ap=[[0, 64]] + list(rd.ap[1:])),
                            )
                            nc.vector.tensor_mul(
                                osb[g2][64 * h : 64 * h + 64, qs],
                                av[0:64, :],
                                bc,
                            )
                    # ---- output projection for chunk c (overlaps chunk c+1)
                    with nc.named_scope("proj"):
                        for n in range(8):
                            acc = atps.tile([128, 512], F32, tag="pp", bufs=1,
                                            name="pacc")
                            for k in range(4):
                                nc.tensor.matmul(
                                    acc,
                                    wp_sb[k][:, 128 * n : 128 * n + 128],
                                    osb[k][:, qs],
                                    start=(k == 0), stop=(k == 3),
                                )
                            yt = at2sb.tile([128, 512], F32, tag="yt", bufs=4,
                                            name="yt")
                            nc.vector.tensor_copy(yt, acc)
                            nc.sync.dma_start(
                                out=yT[128 * n : 128 * n + 128, qs],
                                in_=yt,
                            )
            at2sb_cm.__exit__(None, None, None)
    return nc


_prog = None


def _get_program():
    global _prog
    if _prog is None:
        _prog = build(bacc.Bacc(None))
        _prog.finalize()
    return _prog


def make_in_maps(x, w_qkv, b_qkv, w_proj):
    x = np.ascontiguousarray(np.asarray(x, np.float32))
    w_qkv = np.asarray(w_qkv, np.float32)
    b_qkv = np.asarray(b_qkv, np.float32)
    w_proj = np.asarray(w_proj, np.float32)
    in_maps = []
    for core in range(8):
        b, g = divmod(core, 2)
        gs = slice(DL * g, DL * g + DL)
        gk = slice(D + DL * g, D + DL * g + DL)
        gv = slice(2 * D + DL * g, 2 * D + DL * g + DL)
        in_maps.append({
            "xbT": np.ascontiguousarray(x[b].T).astype(BFNP),
            "wqb": np.ascontiguousarray(w_qkv[:, gs]).astype(BFNP),
            "wkb": np.ascontiguousarray(w_qkv[:, gk]).astype(BFNP),
            "wvb": np.ascontiguousarray(w_qkv[:, gv]).astype(BFNP),
            "bq": np.ascontiguousarray(b_qkv[gs]),
            "bk": np.ascontiguousarray(b_qkv[gk]),
            "wp": np.ascontiguousarray(w_proj[DL * g : DL * g + DL, :]
                                       ).astype(BFNP),
            "ones8": np.ones((128, 8), BFNP),
        })
    return in_maps


def combine_outputs(results, b_qkv, w_proj, b_proj):
    b_qkv = np.asarray(b_qkv, np.float32)
    w_proj = np.asarray(w_proj, np.float32)
    b_proj = np.asarray(b_proj, np.float32)
    # V bias commutes through softmax (sum p = 1): exact host correction
    corr = (b_qkv[2 * D :].astype(np.float64) @ w_proj.astype(np.float64)
            ).astype(np.float32)
    y = np.empty((B, T, D), np.float32)
    for b in range(B):
        yt = results[2 * b]["yT"] + results[2 * b + 1]["yT"]
        y[b] = yt.T + b_proj + corr
    return y


def kernel(x, w_qkv, b_qkv, w_proj, b_proj, **run_kwargs):
    in_maps = make_in_maps(x, w_qkv, b_qkv, w_proj)
    r = run_bass_kernel_spmd(_get_program(), in_maps,
                             core_ids=list(range(8)), **run_kwargs)
    out = combine_outputs(r.results, b_qkv, w_proj, b_proj)
    kernel.last_result = r
    return out
